# revision 3
# baseline (speedup 1.0000x reference)
"""Distributed ring-attention kernel for Trainium2 (8 NeuronCores, Bass/Tile).

Strategy (seq-parallel attention, full softmax without max-subtraction):
  - Host: transpose/cast inputs to bf16; shard x.T column-wise (seq) across 8 cores.
  - Per core: project Q/K/V for its 512-seq shard; AllGather K^T and V
    across cores; compute full attention for its Q shard over the whole
    4096-length K/V; out-projection; write its y shard.
  - Collective schedule: ncfw boots ~21us in and runs a ~37us comm-init
    barrier, so no gathered data can land before ~85us. The K/V exchange is
    4 combined AllGathers of 2 head-pairs each ([kt | v] packed in one flat
    buffer per op), sized so the supply stays just ahead of the two exp
    engines' combined consumption (~0.22M elem/us).
  - Scores are computed transposed (S^T = K @ Q^T, kpos on partitions) so the
    exp'd probabilities feed the P@V matmul directly as the stationary-side
    contraction; head pairs ride the PE array row groups (tile_position) so
    the two 64-deep score matmuls run concurrently. Softmax denominator
    comes from a ones-column appended to V. Softmax skips max-subtraction:
    scores are O(1) here, so exp is numerically safe.
  - exp is SPLIT across two engines: ScalarE runs native exp on ~2/3 of the
    score groups; the DVE handles the rest with a Schraudolph-style bit
    trick: bf16_bits = int16(s * 128*log2e/8 + b), written through an int16
    bitcast of the bf16 tile (one tensor_scalar op per group). The constant
    bias of the trick cancels in softmax; only the sawtooth residual
    (~1.6e-2 worst case full-DVE, ~1.3e-2 at 1/3 share) remains.
"""

import numpy as np
import ml_dtypes

HID = 1024
HEADS = 16
HD = 64
S = 4096
NCORES = 8
SQ = S // NCORES          # 512 q rows per core
PAIRS = HEADS // 2        # 8 head pairs (128 rows of qkvT per pair)
KTILES = S // 128         # 32 kpos tiles per head
VAUG = HD + 1             # 65: V plus ones column
SCALE = 1.0 / np.sqrt(HD)

# Schraudolph exp in bf16-bit space: bits16 = trunc(a*s + b) approximates
# bf16(exp(s/8)) bit pattern.  a = 128*log2(e)*SCALE;  b = 128*(127-sigma)+0.5
SCH_A = 128.0 * 1.4426950408889634 * SCALE
SCH_B = 16245.48

_cache = {}


def _build():
    import concourse.bass as bass
    import concourse.mybir as mybir
    import concourse.tile as tile
    from concourse import bacc

    dt = mybir.dt
    nc = bacc.Bacc("TRN2", target_bir_lowering=False, debug=False,
                   num_devices=NCORES)

    xT = nc.dram_tensor("xT", [HID, SQ], dt.bfloat16, kind="ExternalInput").ap()
    wqkvT = nc.dram_tensor("wqkvT", [HID, 3 * HID], dt.bfloat16,
                           kind="ExternalInput").ap()
    woutT = nc.dram_tensor("woutT", [HID, HID], dt.bfloat16,
                           kind="ExternalInput").ap()
    y = nc.dram_tensor("y", [SQ, HID], dt.float32, kind="ExternalOutput").ap()

    with tile.TileContext(nc) as tc:
        _body(nc, tc, bass, mybir, xT, wqkvT, woutT, y)

    nc.compile()
    return nc


def _body(nc, tc, bass, mybir, xT, wqkvT, woutT, y):
    dt = mybir.dt
    f32, bf16 = dt.float32, dt.bfloat16
    RG = [list(range(NCORES))]

    with (
        tc.tile_pool(name="dram", bufs=1, space="DRAM") as dram,
        tc.tile_pool(name="resident", bufs=1) as res,
        tc.tile_pool(name="stream", bufs=1) as st,
    ):
        # ---- DRAM bounce buffers: one combined [kt | v] buffer per UNIT of
        # 2 head pairs -> 4 AllGathers of 4MB output each, paced to feed the
        # exp engines just-in-time ----
        UNITS = [[0, 1], [2, 3], [4, 5], [6, 7]]
        NU = 2                      # pairs per unit
        unit_of = {}
        for u, prs in enumerate(UNITS):
            for i, p in enumerate(prs):
                unit_of[p] = (u, i)
        USZ = 2 * NU * 128 * SQ     # kt section + v section, elements
        ktvb = [dram.tile([1, USZ], bf16, name=f"ktvb{u}")
                for u in range(len(UNITS))]
        ktvg = [dram.tile([NCORES, USZ], bf16, addr_space="Shared",
                          name=f"ktvg{u}")
                for u in range(len(UNITS))]

        # ---- load xT (hidden x local-seq), 8 resident tiles ----
        xt = []
        for k in range(8):
            t = res.tile([128, SQ], bf16, tag=f"xt{k}", name=f"xt{k}")
            nc.sync.dma_start(t[:], xT[k * 128:(k + 1) * 128, :])
            xt.append(t)

        # wqkvT strip views for batched weight loads
        wq4 = wqkvT.rearrange("(k p) (m c) -> p m k c", p=128, c=128)
        wv2 = wqkvT.rearrange("(k p) (m c) -> p m k c", p=128, c=128)

        def kt_proj(m, psP):
            """K^T rows for pair m (qkvT rows 1024+m*128) -> its unit's
            bounce buffer."""
            u, i = unit_of[m]
            ws = st.tile([128, 8 * 128], bf16, tag="wl", bufs=4)
            nc.sync.dma_start(ws.rearrange("p (k c) -> p k c", c=128),
                              wq4[:, 8 + m, :, :])
            ps = psP.tile([128, SQ], f32, tag="proj", bufs=4)
            for k in range(8):
                nc.tensor.matmul(ps[:], ws[:, k * 128:(k + 1) * 128],
                                 xt[k][:], start=(k == 0), stop=(k == 7))
            sb = st.tile([128, SQ], bf16, tag="kt_stage", bufs=4)
            nc.vector.tensor_copy(sb[:], ps[:])
            ktpart = ktvb[u].rearrange("one (two i r q) -> one two i r q",
                                       two=2, i=NU, r=128, q=SQ)
            nc.sync.dma_start(ktpart[0, 0, i], sb[:])

        def v_proj(u, psP):
            """V rows (natural [s, (i hh d)]) for unit u's 2 pairs -> bounce.

            One psum group per s-tile of 128; output 2*128 = 256 wide."""
            prs = UNITS[u]
            n = NU
            wvs = st.tile([128, 8 * n * 128], bf16, tag="wvs", bufs=3)
            wvs3 = wvs.rearrange("p (k c) -> p k c", c=n * 128)
            nc.sync.dma_start(
                wvs3.rearrange("p k (pr c) -> p k pr c", c=128),
                wv2[:, 16 + prs[0]:16 + prs[0] + n, :, :].rearrange(
                    "p pr k c -> p k pr c"))
            vpart = ktvb[u].rearrange("one (two tl q i c) -> one two tl q i c",
                                      two=2, tl=4, q=128, i=NU, c=128)
            for sti in range(4):
                ps = psP.tile([128, n * 128], f32, tag="proj", bufs=4)
                for k in range(8):
                    nc.tensor.matmul(
                        ps[:], xt[k][:, sti * 128:(sti + 1) * 128],
                        wvs[:, k * n * 128:(k + 1) * n * 128],
                        start=(k == 0), stop=(k == 7))
                sb = st.tile([128, n * 128], bf16, tag="kv_stage", bufs=4)
                nc.vector.tensor_copy(sb[:], ps[:])
                nc.sync.dma_start(
                    vpart[0, 1, sti].rearrange("q i c -> q (i c)"), sb[:])
            nc.gpsimd.collective_compute(
                "AllGather", mybir.AluOpType.bypass, replica_groups=RG,
                ins=[ktvb[u].opt()], outs=[ktvg[u].opt()])

        qt = [None] * PAIRS

        def q_proj(m, psP):
            ws = st.tile([128, 8 * 128], bf16, tag="wl", bufs=4)
            nc.sync.dma_start(ws.rearrange("p (k c) -> p k c", c=128),
                              wq4[:, m, :, :])
            ps = psP.tile([128, SQ], f32, tag="proj", bufs=4)
            for k in range(8):
                nc.tensor.matmul(ps[:], ws[:, k * 128:(k + 1) * 128],
                                 xt[k][:], start=(k == 0), stop=(k == 7))
            t = res.tile([128, SQ], bf16, tag=f"qt{m}", name=f"qt{m}")
            nc.vector.tensor_copy(t[:], ps[:])
            qt[m] = t

        with tc.tile_pool(name="psP", bufs=1, space="PSUM") as psP:
            # per unit: stage kt for both pairs, then v, then fire the
            # combined AllGather; collectives run serially on gpsimd so the
            # emission order is the wire order
            for u in range(len(UNITS)):
                for p in UNITS[u]:
                    kt_proj(p, psP)
                v_proj(u, psP)
            for m in range(PAIRS):
                q_proj(m, psP)

        # ---- attention (head pairs row-packed on the PE array) ----
        attn = []
        for p in range(PAIRS):
            t = res.tile([128, SQ], bf16, tag=f"attn{p}", name=f"attn{p}")
            attn.append(t)

        # out-projection weights (pair-stacked rows: odd heads at
        # partitions 64..127)
        wo5 = woutT.rearrange("(pp r) (o c) -> r o pp c", r=128, c=512)
        wo = []
        for och in range(2):
            w = res.tile([128, PAIRS * 512], bf16, tag=f"wo{och}",
                         name=f"wo{och}")
            nc.sync.dma_start(
                w.rearrange("r (pp c) -> r pp c", c=512), wo5[:, och])
            wo.append(w)

        # exp engine split: every 3rd 3-slot group goes to the DVE via the
        # Schraudolph bit trick (int16 write into the bf16 tile)
        gctr = [0]

        def emit_exp(pt, sc, gw):
            g = gctr[0]
            gctr[0] += 1
            if g % 3 == 2:
                nc.vector.tensor_scalar(
                    pt[:, 0:gw].bitcast(mybir.dt.int16), sc[:, 0:gw],
                    float(SCH_A), float(SCH_B),
                    mybir.AluOpType.mult, mybir.AluOpType.add)
            else:
                nc.scalar.activation(pt[:, 0:gw], sc[:, 0:gw],
                                     mybir.ActivationFunctionType.Exp,
                                     scale=float(SCALE))

        with tc.tile_pool(name="psA", bufs=1, space="PSUM") as psA:
            for u in range(len(UNITS)):
                for i, p in enumerate(UNITS[u]):
                    # pair K^T strip [128, 4096]: rows 0..63 head 2p,
                    # 64..127 head 2p+1
                    ktg3 = ktvg[u].rearrange(
                        "c (two i r q) -> two i r c q",
                        two=2, i=NU, r=128, q=SQ)[0, i]
                    vg4 = ktvg[u].rearrange(
                        "c (two tl q i hh d) -> two i hh q c tl d",
                        two=2, tl=4, q=128, i=NU, hh=2, d=HD)[1, i]
                    kth = st.tile([128, S], bf16, tag="kth", bufs=2)
                    nc.sync.dma_start(
                        kth.rearrange("r (c q) -> r c q", q=SQ), ktg3)
                    vah = []
                    for e in range(2):
                        # data DMA fills cols 0..63 of each 65-wide block;
                        # col 64 is the ones column (strided memset only)
                        va = st.tile([128, KTILES * VAUG], bf16, tag="vah",
                                     bufs=4)
                        va4 = va.rearrange("q (c tl v) -> q c tl v",
                                           tl=4, v=VAUG)
                        nc.vector.memset(va4[:, :, :, HD], 1.0)
                        for tl in range(4):
                            eng = nc.gpsimd if tl % 2 == e else nc.sync
                            eng.dma_start(va4[:, :, tl, 0:HD],
                                          vg4[e, :, :, tl])
                        vah.append(va)

                    pv = [psA.tile([128, 512], f32, tag="pv", bufs=2,
                                   name=f"pv{p}_{e}") for e in range(2)]

                    # slot stream: (t, even), (t, odd) pairs; exp groups
                    # of 3.  PV matmuls lag RA groups behind scores so the
                    # in-order PE queue always has independent work.
                    RA = 3
                    slots = [(t, e) for t in range(KTILES) for e in range(2)]
                    groups = [slots[gs:gs + 3]
                              for gs in range(0, len(slots), 3)]
                    pts = []

                    def emit_scores(group, kth=kth, p=p):
                        gw = 512 * len(group)
                        sc = psA.tile([128, 1536], f32, tag="sc", bufs=2)
                        for idx, (t, e) in enumerate(group):
                            nc.tensor.matmul(
                                sc[:, idx * 512:(idx + 1) * 512],
                                kth[e * 64:(e + 1) * 64,
                                    t * 128:(t + 1) * 128],
                                qt[p][e * 64:(e + 1) * 64, :],
                                start=True, stop=True,
                                tile_position=(e * 64, 0))
                        pt = st.tile([128, 1536], bf16, tag="pt", bufs=RA + 2)
                        emit_exp(pt, sc, gw)
                        pts.append(pt)

                    def emit_pv(group, pt, vah=vah, pv=pv):
                        for idx, (t, e) in enumerate(group):
                            nc.tensor.matmul(
                                pv[e][0:VAUG, :],
                                vah[e][:, t * VAUG:(t + 1) * VAUG],
                                pt[:, idx * 512:(idx + 1) * 512],
                                start=(t == 0), stop=(t == KTILES - 1))

                    for gi, group in enumerate(groups):
                        emit_scores(group)
                        if gi >= RA:
                            emit_pv(groups[gi - RA], pts[gi - RA])
                    for gi in range(len(groups) - RA, len(groups)):
                        emit_pv(groups[gi], pts[gi])

                    # normalize: out_head = pv_data / l (l = row 64).
                    # Evacuate pv to SBUF right away so the PSUM slots free
                    # for the next pair.
                    for e in range(2):
                        pvs = st.tile([VAUG, 512], f32, tag="pvs", bufs=6)
                        nc.vector.tensor_copy(pvs[:], pv[e][0:VAUG, :])
                        l0 = st.tile([1, 512], f32, tag="l0", bufs=2)
                        nc.sync.dma_start(l0[:], pvs[64:65, :])
                        lb = st.tile([64, 512], f32, tag="lb", bufs=2)
                        nc.gpsimd.partition_broadcast(lb[:], l0[:])
                        rb = st.tile([64, 512], f32, tag="rb", bufs=2)
                        nc.vector.reciprocal_approx_fast(rb[:], lb[:])
                        if e == 0:
                            nc.vector.tensor_mul(attn[p][0:64, :],
                                                 pvs[0:64, :], rb[:])
                        else:
                            ao = st.tile([64, SQ], bf16, tag="ao", bufs=2)
                            nc.vector.tensor_mul(ao[:], pvs[0:64, :], rb[:])
                            nc.gpsimd.dma_start(attn[p][64:128, :], ao[:])

        # ---- out projection: y[s, o] = sum_h attn_h^T.T @ woutT[h rows] ----
        with tc.tile_pool(name="psY", bufs=1, space="PSUM") as psY:
            for sti in range(4):
                for och in range(2):
                    psa = psY.tile([128, 512], f32, tag="ya", bufs=4)
                    for p in range(PAIRS):
                        nc.tensor.matmul(
                            psa[:], attn[p][:, sti * 128:(sti + 1) * 128],
                            wo[och][:, p * 512:(p + 1) * 512],
                            start=(p == 0), stop=(p == PAIRS - 1))
                    ysb = st.tile([128, 512], f32, tag="ysb", bufs=4)
                    nc.vector.tensor_copy(ysb[:], psa[:])
                    nc.sync.dma_start(
                        y[sti * 128:(sti + 1) * 128,
                          och * 512:(och + 1) * 512], ysb[:])


def _get_nc():
    if "nc" not in _cache:
        _cache["nc"] = _build()
    return _cache["nc"]


def kernel(x, W_qkv, W_out, _trace=False):
    from concourse.bass_utils import run_bass_kernel_spmd

    nc = _get_nc()
    bf16 = ml_dtypes.bfloat16

    x = np.asarray(x)
    xTf = np.ascontiguousarray(x.reshape(S, HID).T).astype(bf16)   # [HID, S]
    wqkvT = np.ascontiguousarray(np.asarray(W_qkv).T).astype(bf16)
    woutT = np.ascontiguousarray(np.asarray(W_out).T).astype(bf16)

    in_maps = []
    for c in range(NCORES):
        in_maps.append({
            "xT": np.ascontiguousarray(xTf[:, c * SQ:(c + 1) * SQ]),
            "wqkvT": wqkvT,
            "woutT": woutT,
        })
    res = run_bass_kernel_spmd(nc, in_maps, core_ids=list(range(NCORES)),
                               trace=_trace)
    out = np.concatenate([res.results[c]["y"] for c in range(NCORES)],
                         axis=0)
    out = out.reshape(1, S, HID).astype(np.float32)
    if _trace:
        kernel.last_results = res
    return out


# revision 7
# speedup vs baseline: 1.0099x; 1.0099x over previous
"""Distributed ring-attention kernel for Trainium2 (8 NeuronCores, Bass/Tile).

Strategy (seq-parallel attention, full softmax without max-subtraction):
  - Host: transpose/cast inputs to bf16; shard x.T column-wise (seq) across 8 cores.
  - Per core: project Q/K/V for its 512-seq shard; AllGather K^T and V
    across cores; compute full attention for its Q shard over the whole
    4096-length K/V; out-projection; write its y shard.
  - Collective schedule: ncfw boots ~21us in and runs a ~37us comm-init
    barrier, so no gathered data can land before ~85us. The K/V exchange is
    4 combined AllGathers of 2 head-pairs each ([kt | v] packed in one flat
    buffer per op), sized so the supply stays just ahead of the two exp
    engines' combined consumption (~0.22M elem/us).
  - Scores are computed transposed (S^T = K @ Q^T, kpos on partitions) so the
    exp'd probabilities feed the P@V matmul directly as the stationary-side
    contraction; head pairs ride the PE array row groups (tile_position) so
    the two 64-deep score matmuls run concurrently. Softmax denominator
    comes from a ones-column appended to V. Softmax skips max-subtraction:
    scores are O(1) here, so exp is numerically safe.
  - exp is SPLIT across two engines: ScalarE runs native exp on ~2/3 of the
    score groups; the DVE handles the rest with a Schraudolph-style bit
    trick: bf16_bits = int16(s * 128*log2e/8 + b), written through an int16
    bitcast of the bf16 tile (one tensor_scalar op per group). The constant
    bias of the trick cancels in softmax; only the sawtooth residual
    (~1.6e-2 worst case full-DVE, ~1.3e-2 at 1/3 share) remains.
"""

import numpy as np
import ml_dtypes

HID = 1024
HEADS = 16
HD = 64
S = 4096
NCORES = 8
SQ = S // NCORES          # 512 q rows per core
PAIRS = HEADS // 2        # 8 head pairs (128 rows of qkvT per pair)
KTILES = S // 128         # 32 kpos tiles per head
VAUG = HD + 1             # 65: V plus ones column
SCALE = 1.0 / np.sqrt(HD)

# Schraudolph exp in bf16-bit space: bits16 = trunc(a*s + b) approximates
# bf16(exp(s/8)) bit pattern.  a = 128*log2(e)*SCALE;  b = 128*(127-sigma)+0.5
SCH_A = 128.0 * 1.4426950408889634 * SCALE
SCH_B = 16245.48

_cache = {}


def _build():
    import concourse.bass as bass
    import concourse.mybir as mybir
    import concourse.tile as tile
    from concourse import bacc

    dt = mybir.dt
    nc = bacc.Bacc("TRN2", target_bir_lowering=False, debug=False,
                   num_devices=NCORES)

    xT = nc.dram_tensor("xT", [HID, SQ], dt.bfloat16, kind="ExternalInput").ap()
    wqkvT = nc.dram_tensor("wqkvT", [HID, 3 * HID], dt.bfloat16,
                           kind="ExternalInput").ap()
    woutT = nc.dram_tensor("woutT", [HID, HID], dt.bfloat16,
                           kind="ExternalInput").ap()
    y = nc.dram_tensor("y", [SQ, HID], dt.float32, kind="ExternalOutput").ap()

    with tile.TileContext(nc) as tc:
        _body(nc, tc, bass, mybir, xT, wqkvT, woutT, y)

    nc.compile()
    return nc


def _body(nc, tc, bass, mybir, xT, wqkvT, woutT, y):
    dt = mybir.dt
    f32, bf16 = dt.float32, dt.bfloat16
    RG = [list(range(NCORES))]

    with (
        tc.tile_pool(name="dram", bufs=1, space="DRAM") as dram,
        tc.tile_pool(name="resident", bufs=1) as res,
        tc.tile_pool(name="stream", bufs=1) as st,
    ):
        # ---- DRAM bounce buffers: one combined [kt | v] buffer per UNIT of
        # 2 head pairs -> 4 AllGathers of 4MB output each, paced to feed the
        # exp engines just-in-time ----
        UNITS = [[0, 1], [2, 3], [4, 5], [6, 7]]
        NU = 2                      # pairs per unit
        unit_of = {}
        for u, prs in enumerate(UNITS):
            for i, p in enumerate(prs):
                unit_of[p] = (u, i)
        USZ = 2 * NU * 128 * SQ     # kt section + v section, elements
        ktvb = [dram.tile([1, USZ], bf16, name=f"ktvb{u}")
                for u in range(len(UNITS))]
        ktvg = [dram.tile([NCORES, USZ], bf16, addr_space="Shared",
                          name=f"ktvg{u}")
                for u in range(len(UNITS))]

        # tiny dummy AllGather with no data dependencies, fired first: it
        # triggers ncfw's comm-init barrier at boot (~7us) on every core, so
        # the barrier's rendezvous never waits on projection staging (the
        # barrier was observed stretching 37->118us when triggers were late)
        dumb = dram.tile([1, 512], bf16, name="dumb")
        dumg = dram.tile([NCORES, 512], bf16, addr_space="Shared",
                         name="dumg")
        nc.gpsimd.collective_compute(
            "AllGather", mybir.AluOpType.bypass, replica_groups=RG,
            ins=[dumb.opt()], outs=[dumg.opt()])

        # ---- load xT (hidden x local-seq), 8 resident tiles ----
        xt = []
        for k in range(8):
            t = res.tile([128, SQ], bf16, tag=f"xt{k}", name=f"xt{k}")
            nc.sync.dma_start(t[:], xT[k * 128:(k + 1) * 128, :])
            xt.append(t)

        # wqkvT strip views for batched weight loads
        wq4 = wqkvT.rearrange("(k p) (m c) -> p m k c", p=128, c=128)
        wv2 = wqkvT.rearrange("(k p) (m c) -> p m k c", p=128, c=128)

        def kt_proj(m, psP):
            """K^T rows for pair m (qkvT rows 1024+m*128) -> its unit's
            bounce buffer."""
            u, i = unit_of[m]
            ws = st.tile([128, 8 * 128], bf16, tag="wl", bufs=4)
            nc.sync.dma_start(ws.rearrange("p (k c) -> p k c", c=128),
                              wq4[:, 8 + m, :, :])
            ps = psP.tile([128, SQ], f32, tag="proj", bufs=4)
            for k in range(8):
                nc.tensor.matmul(ps[:], ws[:, k * 128:(k + 1) * 128],
                                 xt[k][:], start=(k == 0), stop=(k == 7))
            sb = st.tile([128, SQ], bf16, tag="kt_stage", bufs=4)
            nc.vector.tensor_copy(sb[:], ps[:])
            ktpart = ktvb[u].rearrange("one (two i r q) -> one two i r q",
                                       two=2, i=NU, r=128, q=SQ)
            nc.sync.dma_start(ktpart[0, 0, i], sb[:])

        def v_proj(u, psP):
            """V rows (natural [s, (i hh d)]) for unit u's 2 pairs -> bounce.

            One psum group per s-tile of 128; output 2*128 = 256 wide."""
            prs = UNITS[u]
            n = NU
            wvs = st.tile([128, 8 * n * 128], bf16, tag="wvs", bufs=3)
            wvs3 = wvs.rearrange("p (k c) -> p k c", c=n * 128)
            nc.sync.dma_start(
                wvs3.rearrange("p k (pr c) -> p k pr c", c=128),
                wv2[:, 16 + prs[0]:16 + prs[0] + n, :, :].rearrange(
                    "p pr k c -> p k pr c"))
            vpart = ktvb[u].rearrange("one (two tl q i c) -> one two tl q i c",
                                      two=2, tl=4, q=128, i=NU, c=128)
            for sti in range(4):
                ps = psP.tile([128, n * 128], f32, tag="proj", bufs=4)
                for k in range(8):
                    nc.tensor.matmul(
                        ps[:], xt[k][:, sti * 128:(sti + 1) * 128],
                        wvs[:, k * n * 128:(k + 1) * n * 128],
                        start=(k == 0), stop=(k == 7))
                sb = st.tile([128, n * 128], bf16, tag="kv_stage", bufs=4)
                nc.vector.tensor_copy(sb[:], ps[:])
                nc.sync.dma_start(
                    vpart[0, 1, sti].rearrange("q i c -> q (i c)"), sb[:])
            nc.gpsimd.collective_compute(
                "AllGather", mybir.AluOpType.bypass, replica_groups=RG,
                ins=[ktvb[u].opt()], outs=[ktvg[u].opt()])

        qt = [None] * PAIRS

        def q_proj(m, psP):
            ws = st.tile([128, 8 * 128], bf16, tag="wl", bufs=4)
            nc.sync.dma_start(ws.rearrange("p (k c) -> p k c", c=128),
                              wq4[:, m, :, :])
            ps = psP.tile([128, SQ], f32, tag="proj", bufs=4)
            for k in range(8):
                nc.tensor.matmul(ps[:], ws[:, k * 128:(k + 1) * 128],
                                 xt[k][:], start=(k == 0), stop=(k == 7))
            t = res.tile([128, SQ], bf16, tag=f"qt{m}", name=f"qt{m}")
            nc.vector.tensor_copy(t[:], ps[:])
            qt[m] = t

        with tc.tile_pool(name="psP", bufs=1, space="PSUM") as psP:
            # per unit: stage kt for both pairs, then v, then fire the
            # combined AllGather; collectives run serially on gpsimd so the
            # emission order is the wire order.  q_proj 2..7 are deferred
            # into the gather window to keep the PE warm (HAM throttle).
            for u in range(len(UNITS)):
                for p in UNITS[u]:
                    kt_proj(p, psP)
                v_proj(u, psP)
            q_proj(0, psP)
            q_proj(1, psP)

        # ---- attention (head pairs row-packed on the PE array) ----
        attn = []
        for p in range(PAIRS):
            t = res.tile([128, SQ], bf16, tag=f"attn{p}", name=f"attn{p}")
            attn.append(t)

        # out-projection weights (pair-stacked rows: odd heads at
        # partitions 64..127)
        wo5 = woutT.rearrange("(pp r) (o c) -> r o pp c", r=128, c=512)
        wo = []
        for och in range(2):
            w = res.tile([128, PAIRS * 512], bf16, tag=f"wo{och}",
                         name=f"wo{och}")
            nc.sync.dma_start(
                w.rearrange("r (pp c) -> r pp c", c=512), wo5[:, och])
            wo.append(w)

        # exp engine split: every 3rd 2-slot group goes to the DVE via the
        # Schraudolph bit trick (int16 write into the bf16 tile)
        gctr = [0]

        def emit_exp(pt, sc, gw):
            g = gctr[0]
            gctr[0] += 1
            if g % 3 == 2:
                nc.vector.tensor_scalar(
                    pt[:, 0:gw].bitcast(mybir.dt.int16), sc[:, 0:gw],
                    float(SCH_A), float(SCH_B),
                    mybir.AluOpType.mult, mybir.AluOpType.add)
            else:
                nc.scalar.activation(pt[:, 0:gw], sc[:, 0:gw],
                                     mybir.ActivationFunctionType.Exp,
                                     scale=float(SCALE))

        # deferred q projections run in the gather window (own pool scope,
        # sequential with psP/psA)
        with tc.tile_pool(name="psQ", bufs=1, space="PSUM") as psQ:
            for m in range(2, PAIRS):
                q_proj(m, psQ)

        with tc.tile_pool(name="psA", bufs=1, space="PSUM") as psA:
            # PE warm-keeper chain through the gather window: tiny
            # matmul->copy->matmul links paced by the cross-engine semaphore
            # latency keep HAM from re-throttling the PE during the
            # otherwise idle 60..100us stretch.
            wk = st.tile([1, 64], bf16, tag="wk", bufs=2)
            nc.vector.tensor_copy(wk[:], xt[0][0:1, 0:64])
            for _ in range(35):
                ps = psA.tile([128, 1024], f32, tag="sc", bufs=3)
                nc.tensor.matmul(ps[0:1, 0:64], wk[0:1, 0:1],
                                 wk[0:1, 0:64], start=True, stop=True)
                wk = st.tile([1, 64], bf16, tag="wk", bufs=2)
                nc.vector.tensor_copy(wk[:], ps[0:1, 0:64])

            for u in range(len(UNITS)):
                kth_u, vah_u = [], []
                for i, p in enumerate(UNITS[u]):
                    # pair K^T strip [128, 4096]: rows 0..63 head 2p,
                    # 64..127 head 2p+1; prefetch both pairs' K/V loads as
                    # soon as the unit's gather lands
                    ktg3 = ktvg[u].rearrange(
                        "c (two i r q) -> two i r c q",
                        two=2, i=NU, r=128, q=SQ)[0, i]
                    vg4 = ktvg[u].rearrange(
                        "c (two tl q i hh d) -> two i hh q c tl d",
                        two=2, tl=4, q=128, i=NU, hh=2, d=HD)[1, i]
                    kth = st.tile([128, S], bf16, tag="kth", bufs=2)
                    nc.sync.dma_start(
                        kth.rearrange("r (c q) -> r c q", q=SQ), ktg3)
                    kth_u.append(kth)
                    vah = []
                    for e in range(2):
                        # data DMA fills cols 0..63 of each 65-wide block;
                        # col 64 is the ones column (strided memset only)
                        va = st.tile([128, KTILES * VAUG], bf16, tag="vah",
                                     bufs=4)
                        va4 = va.rearrange("q (c tl v) -> q c tl v",
                                           tl=4, v=VAUG)
                        nc.vector.memset(va4[:, :, :, HD], 1.0)
                        for tl in range(4):
                            eng = nc.gpsimd if tl % 2 == e else nc.sync
                            eng.dma_start(va4[:, :, tl, 0:HD],
                                          vg4[e, :, :, tl])
                        vah.append(va)
                    vah_u.append(vah)

                for i, p in enumerate(UNITS[u]):
                    kth, vah = kth_u[i], vah_u[i]
                    pv = [psA.tile([128, 512], f32, tag="pv", bufs=2,
                                   name=f"pv{p}_{e}") for e in range(2)]

                    # slot stream: (t, even), (t, odd); 2-slot groups so the
                    # two scores matmuls are always a concurrent row pair.
                    # PV matmuls lag RA groups behind scores so the in-order
                    # PE queue always has independent work.
                    RA = 4
                    slots = [(t, e) for t in range(KTILES) for e in range(2)]
                    groups = [slots[gs:gs + 2]
                              for gs in range(0, len(slots), 2)]
                    pts = []

                    def emit_scores(group, kth=kth, p=p):
                        gw = 512 * len(group)
                        sc = psA.tile([128, 1024], f32, tag="sc", bufs=3)
                        for idx, (t, e) in enumerate(group):
                            nc.tensor.matmul(
                                sc[:, idx * 512:(idx + 1) * 512],
                                kth[e * 64:(e + 1) * 64,
                                    t * 128:(t + 1) * 128],
                                qt[p][e * 64:(e + 1) * 64, :],
                                start=True, stop=True,
                                tile_position=(e * 64, 0))
                        pt = st.tile([128, 1024], bf16, tag="pt", bufs=RA + 2)
                        emit_exp(pt, sc, gw)
                        pts.append(pt)

                    def emit_pv(group, pt, vah=vah, pv=pv):
                        for idx, (t, e) in enumerate(group):
                            nc.tensor.matmul(
                                pv[e][0:VAUG, :],
                                vah[e][:, t * VAUG:(t + 1) * VAUG],
                                pt[:, idx * 512:(idx + 1) * 512],
                                start=(t == 0), stop=(t == KTILES - 1))

                    for gi, group in enumerate(groups):
                        emit_scores(group)
                        if gi >= RA:
                            emit_pv(groups[gi - RA], pts[gi - RA])
                    for gi in range(len(groups) - RA, len(groups)):
                        emit_pv(groups[gi], pts[gi])

                    # normalize: out_head = pv_data / l (l = row 64).
                    # Evacuate pv to SBUF right away so the PSUM slots free
                    # for the next pair.
                    for e in range(2):
                        pvs = st.tile([VAUG, 512], f32, tag="pvs", bufs=6)
                        nc.vector.tensor_copy(pvs[:], pv[e][0:VAUG, :])
                        l0 = st.tile([1, 512], f32, tag="l0", bufs=2)
                        nc.gpsimd.dma_start(l0[:], pvs[64:65, :])
                        lb = st.tile([64, 512], f32, tag="lb", bufs=2)
                        nc.gpsimd.partition_broadcast(lb[:], l0[:])
                        rb = st.tile([64, 512], f32, tag="rb", bufs=2)
                        nc.vector.reciprocal_approx_fast(rb[:], lb[:])
                        if e == 0:
                            nc.vector.tensor_mul(attn[p][0:64, :],
                                                 pvs[0:64, :], rb[:])
                        else:
                            ao = st.tile([64, SQ], bf16, tag="ao", bufs=2)
                            nc.vector.tensor_mul(ao[:], pvs[0:64, :], rb[:])
                            nc.gpsimd.dma_start(attn[p][64:128, :], ao[:])

        # ---- out projection: y[s, o] = sum_h attn_h^T.T @ woutT[h rows] ----
        with tc.tile_pool(name="psY", bufs=1, space="PSUM") as psY:
            for sti in range(4):
                for och in range(2):
                    psa = psY.tile([128, 512], f32, tag="ya", bufs=4)
                    for p in range(PAIRS):
                        nc.tensor.matmul(
                            psa[:], attn[p][:, sti * 128:(sti + 1) * 128],
                            wo[och][:, p * 512:(p + 1) * 512],
                            start=(p == 0), stop=(p == PAIRS - 1))
                    ysb = st.tile([128, 512], f32, tag="ysb", bufs=4)
                    nc.vector.tensor_copy(ysb[:], psa[:])
                    nc.sync.dma_start(
                        y[sti * 128:(sti + 1) * 128,
                          och * 512:(och + 1) * 512], ysb[:])


def _get_nc():
    if "nc" not in _cache:
        _cache["nc"] = _build()
    return _cache["nc"]


def kernel(x, W_qkv, W_out, _trace=False):
    from concourse.bass_utils import run_bass_kernel_spmd

    nc = _get_nc()
    bf16 = ml_dtypes.bfloat16

    x = np.asarray(x)
    xTf = np.ascontiguousarray(x.reshape(S, HID).T).astype(bf16)   # [HID, S]
    wqkvT = np.ascontiguousarray(np.asarray(W_qkv).T).astype(bf16)
    woutT = np.ascontiguousarray(np.asarray(W_out).T).astype(bf16)

    in_maps = []
    for c in range(NCORES):
        in_maps.append({
            "xT": np.ascontiguousarray(xTf[:, c * SQ:(c + 1) * SQ]),
            "wqkvT": wqkvT,
            "woutT": woutT,
        })
    res = run_bass_kernel_spmd(nc, in_maps, core_ids=list(range(NCORES)),
                               trace=_trace)
    out = np.concatenate([res.results[c]["y"] for c in range(NCORES)],
                         axis=0)
    out = out.reshape(1, S, HID).astype(np.float32)
    if _trace:
        kernel.last_results = res
    return out
